# revision 1
# baseline (speedup 1.0000x reference)
"""Self-contained 8-core data-parallel kernel for nn_AspectSent.

Shards B=256 across the 8 NeuronCores (32 samples/core, params
replicated), runs the full GRU+CRF forward per shard on-device, and
combines the tiny per-core partial sums on host.
"""
import numpy as np
import jax
import jax.numpy as jnp
from jax import lax

B, T, V = 256, 128, 30000
EMB, MDIM = 300, 50
H = 1024
HD = H // 2
L = 4
C1, C2 = 1.0, 1.0
NCORES = 8
BS = B // NCORES  # 32 per core

PARAM_NAMES = [
    'word_embed', 'mask_embed',
    'w_ih_f', 'w_hh_f', 'b_ih_f', 'b_hh_f',
    'w_ih_b', 'w_hh_b', 'b_ih_b', 'b_hh_b',
    'tri_w', 'tri_b', 'trans', 'sent_w', 'sent_b',
]


def _gru_scan(x, m, w_ih, w_hh, b_ih, b_hh):
    xg = jnp.einsum('btd,gd->btg', x, w_ih) + b_ih

    def step(h, inp):
        xg_t, m_t = inp
        hg = h @ w_hh.T + b_hh
        xr, xz, xn = jnp.split(xg_t, 3, axis=-1)
        hr, hz, hn = jnp.split(hg, 3, axis=-1)
        r = jax.nn.sigmoid(xr + hr)
        z = jax.nn.sigmoid(xz + hz)
        n = jnp.tanh(xn + r * hn)
        h_new = (1.0 - z) * n + z * h
        h = jnp.where(m_t[:, None] > 0, h_new, h)
        return h, h * m_t[:, None]

    h0 = jnp.zeros((x.shape[0], HD), x.dtype)
    _, out = lax.scan(step, h0, (xg.transpose(1, 0, 2), m.T))
    return out.transpose(1, 0, 2)


def _rev_valid(x, lens):
    idx = lens[:, None] - 1 - jnp.arange(x.shape[1])
    valid = idx >= 0
    g = jnp.take_along_axis(x, jnp.clip(idx, 0, x.shape[1] - 1)[:, :, None], axis=1)
    return jnp.where(valid[:, :, None], g, 0.0)


def _crf_marginals(feats, wmask, trans):
    f_t = feats.transpose(1, 0, 2)
    m_t = wmask.T

    def fstep(alpha, inp):
        f, m = inp
        a_new = f + jax.nn.logsumexp(alpha[:, :, None] + trans[None], axis=1)
        alpha = jnp.where(m[:, None] > 0, a_new, alpha)
        return alpha, alpha

    a0 = f_t[0]
    _, alphas = lax.scan(fstep, a0, (f_t[1:], m_t[1:]))
    alphas = jnp.concatenate([a0[None], alphas], axis=0)

    def bstep(beta, inp):
        f, m = inp
        b_new = jax.nn.logsumexp(trans[None] + (f + beta)[:, None, :], axis=2)
        beta = jnp.where(m[:, None] > 0, b_new, beta)
        return beta, beta

    b_last = jnp.zeros_like(a0)
    _, betas = lax.scan(bstep, b_last, (f_t[1:][::-1], m_t[1:][::-1]))
    betas = jnp.concatenate([betas[::-1], b_last[None]], axis=0)
    marg = jax.nn.softmax(alphas + betas, axis=-1)
    return marg.transpose(1, 0, 2) * wmask[:, :, None]


def _shard_forward(sents, masks, labels, lens,
                   word_embed, mask_embed,
                   w_ih_f, w_hh_f, b_ih_f, b_hh_f,
                   w_ih_b, w_hh_b, b_ih_b, b_hh_b,
                   tri_w, tri_b, trans, sent_w, sent_b):
    x = jnp.concatenate([word_embed[sents], mask_embed[masks]], axis=-1)
    wmask = (jnp.arange(T)[None, :] < lens[:, None]).astype(x.dtype)
    out_f = _gru_scan(x, wmask, w_ih_f, w_hh_f, b_ih_f, b_hh_f)
    out_b = _rev_valid(
        _gru_scan(_rev_valid(x, lens), wmask, w_ih_b, w_hh_b, b_ih_b, b_hh_b),
        lens)
    context = jnp.concatenate([out_f, out_b], axis=-1)
    ctx = jnp.tanh(context)
    mf = masks.astype(ctx.dtype)
    tavg = (mf[:, :, None] * ctx).sum(1) / mf.sum(1, keepdims=True)
    ctx = ctx + tavg[:, None, :]
    feats = ctx @ tri_w + tri_b
    marg = _crf_marginals(feats, wmask, trans)
    sp = marg[:, :, 1]
    gamma = sp.sum(1) / 2.0
    sent_vs = jnp.einsum('bt,btd->bd', sp, ctx) / gamma[:, None]
    label_scores = sent_vs @ sent_w + sent_b
    scores = jax.nn.log_softmax(label_scores, axis=1)
    picked = jnp.take_along_axis(scores, labels[:, None], axis=1)[:, 0]
    spsum = sp.sum(1)
    return scores, picked, spsum


_pmapped = None


def _get_pmapped():
    global _pmapped
    if _pmapped is None:
        _pmapped = jax.pmap(
            _shard_forward,
            in_axes=(0, 0, 0, 0) + (None,) * len(PARAM_NAMES),
            devices=jax.devices()[:NCORES],
        )
    return _pmapped


def kernel(**inputs):
    sents = np.asarray(inputs['sents']).astype(np.int32).reshape(NCORES, BS, T)
    masks = np.asarray(inputs['masks']).astype(np.int32).reshape(NCORES, BS, T)
    labels = np.asarray(inputs['labels']).astype(np.int32).reshape(NCORES, BS)
    lens = np.asarray(inputs['lens']).astype(np.int32).reshape(NCORES, BS)
    params = [np.asarray(inputs[k]).astype(np.float32) for k in PARAM_NAMES]

    fn = _get_pmapped()
    scores_sh, picked_sh, spsum_sh = fn(sents, masks, labels, lens, *params)
    scores = np.asarray(scores_sh).reshape(B, 3).astype(np.float32)
    picked = np.asarray(picked_sh).reshape(B)
    spsum = np.asarray(spsum_sh).reshape(B)

    cls_loss = np.float32(-np.mean(picked))
    s_prob_norm = np.mean(spsum)
    trans = np.asarray(inputs['trans']).astype(np.float32)
    pena = max(trans[1, 0] - trans[0, 0], 0.0) + max(trans[0, 1] - trans[1, 1], 0.0)
    norm_pen = np.float32(C1 * pena + C2 * s_prob_norm)
    return cls_loss, norm_pen, scores
